# revision 38
# baseline (speedup 1.0000x reference)
"""GAT + global-max-pool + LSTM + Linear kernel for Trainium2 (8 NeuronCores).

Sharding: data-parallel over the batch axis B=8 -> one sequence b per core.
Each core computes the GAT over its 16 graphs (t=0..15), global-max-pools,
runs the LSTM over its sequence locally, and emits one [8] output row.

v2 redesign (engine-balanced):
  - host pre-transposes x -> xT [16, G*1024]; per-graph DMA load (no PE
    transposes on device).
  - one merged fp32r matmul per 128-src-node block J computes xp|a_src
    directly in [m, hd] layout (no xpT + transpose round trip).
  - per head: raw a_dst row broadcast once to 128 partitions (2 fp32r
    matmuls -> adB PSUM); vb_sb = Exp(adB), v5b_sb = Exp(0.2 adB) SBUF
    mirrors feed the DVE fast paths.
  - dense attention tile per (head h, src block J), engine-split:
      A-path (small J): t1 = ACT Exp(adB + a_s), t5 = ACT Exp(.2 adB + .2 a_s),
                        tM = DVE tt-max
      B-path: tE = DVE ts(vb_sb * u) in 4x mode; tM via Pool fused stt or
              DVE ts+tt
      tA = tM * cnt on DVE (2x bf16) or Pool
      agg: oph[33, n] += xp33^T @ tA   (bf16 matmuls, 500-col halves)
  - divide+relu+maxpool fused and software-pipelined two heads deep:
    Pool copies oph -> SBUF (frees the single PSUM buffer), DMA folds the
    denominator row to [40,25], DVE reciprocal, DMA unfold, bf16 PE
    broadcast of rec row, tensor_tensor_reduce chains the max over both
    halves; b_gat applied after the reduce (exact: b is constant over
    nodes).
  - LSTM step g emitted inside graph g+1 (hidden behind its factor
    stage). Gates packed 2-per-matmul, tanh-sigmoid trick with
    per-partition scale columns.
  - real 1000 cols only, gap layout (halves at col offsets 0 and 512).

softmax max-subtraction is dropped: alpha = ex/sum(ex) is invariant to the
per-dst shift and fp32/bf16 exp() of |z| <~ 10 cannot overflow.
"""

import numpy as np

import concourse.bacc as bacc
import concourse.bass as bass
import concourse.mybir as mybir
import concourse.tile as tile
from concourse.bass_utils import run_bass_kernel_spmd

B, T, N, F_IN = 8, 16, 1000, 16
H, D = 4, 32
HD = H * D          # 128
HL = 64
OUT = 8
NEG = 0.2
NPAD = 1024         # padded node count
NBLK = 8            # src blocks of 128
G = T               # graphs per core
NH = 500            # real cols per half
GAP = 512           # col offset stride of halves
BLKC = H * 33 + 4   # xp33 cols per src block (4 head groups + pad)

FP = mybir.dt.float32
FR = mybir.dt.float32r
BF = mybir.dt.bfloat16
AX = mybir.AxisListType
AF = mybir.ActivationFunctionType
OPS = mybir.AluOpType

_CACHE = {}

# engine assignment knobs, per src block J (same for all heads)
A_PATH_J = 0          # J < this: t1/t5 via ACT Exp-with-bias
P2_POOL_J = ()  # fused stt on Pool
P3_POOL_J = ()     # cnt-mult on Pool


def _rv(ap):
    """Real-column view [p, 2, 500] of a gap-layout [p, 1024] AP."""
    return ap.rearrange("p (a b) -> p a b", b=GAP)[:, :, 0:NH]


def _build_nc():
    nc = bacc.Bacc("TRN2", target_bir_lowering=False, debug=False)

    # ---- DRAM I/O ----
    d_xT = nc.dram_tensor("x_t", [F_IN, G * NPAD], FR, kind="ExternalInput").ap()
    d_wall = nc.dram_tensor("w_all", [F_IN, HD + H], FR, kind="ExternalInput").ap()
    d_wad = nc.dram_tensor("w_ad", [F_IN, H], FR, kind="ExternalInput").ap()
    d_cnt = nc.dram_tensor("cntmask", [128, NBLK * NPAD], BF, kind="ExternalInput").ap()
    d_onesb = nc.dram_tensor("ones_bf", [1, 128], BF, kind="ExternalInput").ap()
    d_onesf = nc.dram_tensor("ones_fp", [1, 128], FR, kind="ExternalInput").ap()
    d_bgat = nc.dram_tensor("b_gat", [32, H], FP, kind="ExternalInput").ap()
    d_wih01 = nc.dram_tensor("wih01", [HD, 2 * HL], FP, kind="ExternalInput").ap()
    d_wih23 = nc.dram_tensor("wih23", [HD, 2 * HL], FP, kind="ExternalInput").ap()
    d_whh01 = nc.dram_tensor("whh01", [HL, 2 * HL], FP, kind="ExternalInput").ap()
    d_whh23 = nc.dram_tensor("whh23", [HL, 2 * HL], FP, kind="ExternalInput").ap()
    d_bls = nc.dram_tensor("b_lstm", [2 * HL, 2], FP, kind="ExternalInput").ap()
    d_scl23 = nc.dram_tensor("scl23", [2 * HL, 1], FP, kind="ExternalInput").ap()
    d_wclf = nc.dram_tensor("wclf_t", [HL, OUT], FP, kind="ExternalInput").ap()
    d_bclf = nc.dram_tensor("b_clf", [OUT, 1], FP, kind="ExternalInput").ap()
    d_y = nc.dram_tensor("y", [OUT, 1], FP, kind="ExternalOutput").ap()
    d_dbg = nc.dram_tensor("dbg_pool", [HD, G], FP, kind="ExternalOutput").ap()
    d_dbg2 = nc.dram_tensor("dbg_osb", [33, NPAD], FP, kind="ExternalOutput").ap()
    d_dbg3 = nc.dram_tensor("dbg_rec", [1, NPAD], BF, kind="ExternalOutput").ap()
    d_dbg4 = nc.dram_tensor("dbg_tA", [128, NPAD], BF, kind="ExternalOutput").ap()
    d_dbg5 = nc.dram_tensor("dbg_vb", [128, NPAD], BF, kind="ExternalOutput").ap()
    d_dbg6 = nc.dram_tensor("dbg_fac", [128, NBLK * 8], FP, kind="ExternalOutput").ap()

    with tile.TileContext(nc) as tc:
        with (
            tc.tile_pool(name="const", bufs=1) as cpool,
            tc.tile_pool(name="xtp", bufs=2) as xtp,
            tc.tile_pool(name="fact", bufs=2) as fpool,
            tc.tile_pool(name="bcs", bufs=2) as bpool,
            tc.tile_pool(name="edense", bufs=3) as epool,
            tc.tile_pool(name="divp", bufs=3) as dpool,
            tc.tile_pool(name="lstm", bufs=2) as lpool,
            tc.tile_pool(name="ps_bc", bufs=1, space="PSUM") as ps_bc,
            tc.tile_pool(name="ps_out", bufs=2, space="PSUM") as ps_out,
            tc.tile_pool(name="ps_misc", bufs=2, space="PSUM") as ps_misc,
        ):
            # ---- load constants ----
            c_wall = cpool.tile([F_IN, HD + H], FR, tag="wall")
            nc.sync.dma_start(c_wall[:], d_wall)
            c_wad = cpool.tile([F_IN, H], FR, tag="wad")
            nc.sync.dma_start(c_wad[:], d_wad)
            c_cnt = cpool.tile([128, NBLK * NPAD], BF, tag="cnt")
            nc.sync.dma_start(c_cnt[:], d_cnt)
            c_onesb = cpool.tile([1, 128], BF, tag="onesb")
            nc.sync.dma_start(c_onesb[:], d_onesb)
            c_onesf = cpool.tile([1, 128], FR, tag="onesf")
            nc.sync.dma_start(c_onesf[:], d_onesf)
            c_bgat = cpool.tile([32, H], FP, tag="bgat")
            nc.sync.dma_start(c_bgat[:], d_bgat)
            c_wih01 = cpool.tile([HD, 2 * HL], FP, tag="wih01")
            nc.sync.dma_start(c_wih01[:], d_wih01)
            c_wih23 = cpool.tile([HD, 2 * HL], FP, tag="wih23")
            nc.sync.dma_start(c_wih23[:], d_wih23)
            c_whh01 = cpool.tile([HL, 2 * HL], FP, tag="whh01")
            nc.sync.dma_start(c_whh01[:], d_whh01)
            c_whh23 = cpool.tile([HL, 2 * HL], FP, tag="whh23")
            nc.sync.dma_start(c_whh23[:], d_whh23)
            c_bls = cpool.tile([2 * HL, 2], FP, tag="bls")
            nc.sync.dma_start(c_bls[:], d_bls)
            c_scl23 = cpool.tile([2 * HL, 1], FP, tag="scl23")
            nc.sync.dma_start(c_scl23[:], d_scl23)
            c_wclf = cpool.tile([HL, OUT], FP, tag="wclf")
            nc.sync.dma_start(c_wclf[:], d_wclf)
            c_bclf = cpool.tile([OUT, 1], FP, tag="bclf")
            nc.sync.dma_start(c_bclf[:], d_bclf)

            # persistent: pooled sequence + manually double-buffered xp33
            c_pool = cpool.tile([HD, G], FP, tag="pooled")
            xp33s = []
            for i in range(2):
                xp_t = cpool.tile([128, NBLK * BLKC], BF, tag=f"xp33_{i}")
                for J in range(NBLK):
                    nc.vector.memset(
                        xp_t[:, J * BLKC:J * BLKC + H * 33].rearrange(
                            "p (h q) -> p h q", q=33
                        )[:, :, 32:33],
                        1.0,
                    )
                xp33s.append(xp_t)

            hprev = lpool.tile([HL, 1], FP, tag="h0")
            cprev = lpool.tile([HL, 1], FP, tag="c0")
            nc.vector.memset(hprev[:], 0.0)
            nc.vector.memset(cprev[:], 0.0)
            lstm_state = [hprev, cprev]

            def emit_lstm(g):
                hp, cp = lstm_state
                psg01 = ps_misc.tile([2 * HL, 1], FP, tag="pm")
                nc.tensor.matmul(
                    psg01[:], c_wih01[:],
                    c_pool[:, g:g + 1], start=True, stop=False,
                )
                nc.tensor.matmul(
                    psg01[:], c_whh01[:], hp[:],
                    start=False, stop=True,
                )
                psg23 = ps_misc.tile([2 * HL, 1], FP, tag="pm")
                nc.tensor.matmul(
                    psg23[:], c_wih23[:],
                    c_pool[:, g:g + 1], start=True, stop=False,
                )
                nc.tensor.matmul(
                    psg23[:], c_whh23[:], hp[:],
                    start=False, stop=True,
                )
                tg01 = lpool.tile([2 * HL, 1], FP, tag="tg01")
                nc.scalar.activation(
                    tg01[:], psg01[:], AF.Tanh, bias=c_bls[:, 0:1], scale=0.5,
                )
                tg23 = lpool.tile([2 * HL, 1], FP, tag="tg23")
                nc.scalar.activation(
                    tg23[:], psg23[:], AF.Tanh, bias=c_bls[:, 1:2],
                    scale=c_scl23[:, 0:1],
                )
                tf0 = lpool.tile([HL, 1], FP, tag="tf0")
                nc.sync.dma_start(tf0[:], tg01[HL:2 * HL, :])
                to0 = lpool.tile([HL, 1], FP, tag="to0")
                nc.sync.dma_start(to0[:], tg23[HL:2 * HL, :])
                # v1 = (tf+1)*c2 ; v2 = (ti+1)*tg ; c2' = v1/2 + v2
                v1 = lpool.tile([HL, 1], FP, tag="v1")
                nc.vector.scalar_tensor_tensor(
                    v1[:], tf0[:], 1.0, cp[:], OPS.add, OPS.mult
                )
                v2 = lpool.tile([HL, 1], FP, tag="v2")
                nc.vector.scalar_tensor_tensor(
                    v2[:], tg01[0:HL, :], 1.0, tg23[0:HL, :], OPS.add, OPS.mult
                )
                cnew = lpool.tile([HL, 1], FP, tag="c0")
                nc.vector.scalar_tensor_tensor(
                    cnew[:], v1[:], 0.5, v2[:], OPS.mult, OPS.add
                )
                tcn = lpool.tile([HL, 1], FP, tag="tcn")
                nc.scalar.activation(tcn[:], cnew[:], AF.Tanh, scale=0.5)
                hnew = lpool.tile([HL, 1], FP, tag="h0")
                nc.vector.scalar_tensor_tensor(
                    hnew[:], to0[:], 1.0, tcn[:], OPS.add, OPS.mult
                )
                lstm_state[0], lstm_state[1] = hnew, cnew

            def emit_bcast(ad_row, h):
                """Broadcast a_d row h to 128 partitions; SBUF mirrors."""
                adB = ps_bc.tile([128, NPAD], FP, tag="adB")
                for half in range(2):
                    sl = slice(half * GAP, half * GAP + NH)
                    nc.tensor.matmul(
                        adB[:, sl], c_onesf[:],
                        ad_row[:, sl],
                        start=True, stop=True,
                    )
                vb_sb = bpool.tile([128, NPAD], BF, tag=f"vb_sb{h}")
                nc.scalar.activation(_rv(vb_sb[:]), _rv(adB[:]), AF.Exp)
                v5b_sb = bpool.tile([128, NPAD], BF, tag=f"v5b_sb{h}")
                nc.scalar.activation(_rv(v5b_sb[:]), _rv(adB[:]), AF.Exp, scale=NEG)
                adB_sb = bpool.tile([128, NPAD], mybir.dt.float16,
                                    tag=f"adB_sb{h}")
                nc.scalar.activation(_rv(adB_sb[:]), _rv(adB[:]), AF.Copy)
                return adB_sb, vb_sb, v5b_sb

            for g in range(G):
                xp33 = xp33s[g % 2]
                # ---- load xT for this graph ----
                xT = xtp.tile([F_IN, NPAD], FR, tag="xT")
                nc.sync.dma_start(xT[:], d_xT[:, g * NPAD:(g + 1) * NPAD])
                xTr = xT[:]

                # ---- a_dst rows (raw, fp32); one [1, NPAD] tile per head
                # (matmul rhs requires base partition 0) ----
                ad_sb = fpool.tile([H, NPAD], FR, tag="ad_sb")
                for half in range(2):
                    pad_ = ps_misc.tile([H, GAP], FP, tag="pm")
                    nc.tensor.matmul(
                        pad_[:, 0:NH], c_wad[:],
                        xTr[:, half * NH:half * NH + NH],
                        start=True, stop=True,
                    )
                    nc.scalar.activation(
                        ad_sb[:, half * GAP:half * GAP + NH],
                        pad_[:, 0:NH], AF.Copy,
                    )
                nc.vector.memset(ad_sb[:, NH:GAP].bitcast(mybir.dt.uint32), 0)
                nc.vector.memset(ad_sb[:, GAP + NH:NPAD].bitcast(mybir.dt.uint32), 0)
                ad_rows = []
                for h in range(H):
                    ad_h = fpool.tile([1, NPAD], FR, tag=f"ad_h{h}")
                    nc.sync.dma_start(ad_h[:], ad_sb[h:h + 1, :])
                    ad_rows.append(ad_h)

                # ---- per src block: xp | a_src factors, plus all four
                # head broadcasts interleaved to keep the PE streaming ----
                # c_fac cols per J: [0:4] = a_s (A-path) or exp(a_s) (B-path)
                #                   [4:8] = 0.2*a_s or exp(0.2*a_s)
                c_fac = fpool.tile([128, NBLK * 8], FP, tag="c_fac")
                bcast = [None] * H

                def emit_pxa(J):
                    pxa = ps_misc.tile([128, HD + H], FP, tag="pm")
                    nc.tensor.matmul(
                        pxa[:], xTr[:, J * 128:(J + 1) * 128],
                        c_wall[:],
                        start=True, stop=True,
                    )
                    base = J * BLKC
                    nc.vector.tensor_copy(
                        xp33[:, base:base + H * 33].rearrange(
                            "p (h q) -> p h q", q=33
                        )[:, :, 0:32],
                        pxa[:, 0:HD].rearrange("p (h q) -> p h q", q=32),
                    )
                    if J < PRELU_J:
                        nc.vector.tensor_copy(
                            c_fac[:, J * 8:J * 8 + 4], pxa[:, HD:HD + H],
                        )
                    else:
                        nc.scalar.activation(
                            c_fac[:, J * 8:J * 8 + 4], pxa[:, HD:HD + H],
                            AF.Exp, scale=1.0,
                        )
                        nc.scalar.activation(
                            c_fac[:, J * 8 + 4:J * 8 + 8], pxa[:, HD:HD + H],
                            AF.Exp, scale=NEG,
                        )

                emit_pxa(0)
                emit_pxa(1)
                for h in range(H):
                    bcast[h] = emit_bcast(ad_rows[h], h)
                    if 2 + h < NBLK:
                        emit_pxa(2 + h)
                emit_pxa(6)
                emit_pxa(7)
                if g > 0:
                    emit_lstm(g - 1)

                # ---- heads: software-pipelined divide path ----
                # stage state carried across head iterations
                pending = {}   # h -> dict of tiles for deferred stages
                for h in range(H):
                    adB_ps, vb_sb, v5b_sb = bcast[h]

                    oph = ps_out.tile([33, NPAD], FP, tag="oph")
                    for J in range(NBLK):
                        fcol = c_fac[:, J * 8 + h:J * 8 + h + 1]
                        f5col = c_fac[:, J * 8 + 4 + h:J * 8 + 5 + h]
                        cntJ = c_cnt[:, J * NPAD:(J + 1) * NPAD]
                        tA = epool.tile([128, NPAD], BF, tag="tA")
                        if J < PRELU_J:
                            # w = lrelu(a_d[n] + a_s[m]); tM = exp(w)
                            tw = epool.tile([128, NPAD], mybir.dt.float16,
                                            tag="tw")
                            nc.scalar.activation(
                                _rv(tw[:]), _rv(adB_ps[:]), AF.Prelu,
                                bias=fcol, scale=1.0, alpha=NEG,
                            )
                            tM = epool.tile([128, NPAD], BF, tag="tM")
                            nc.scalar.activation(
                                _rv(tM[:]), _rv(tw[:]), AF.Exp,
                            )
                            eng = nc.gpsimd if J in P3_POOL_J else nc.vector
                            eng.tensor_tensor(
                                _rv(tA[:]), _rv(tM[:]), _rv(cntJ), OPS.mult,
                            )
                        else:
                            t1 = epool.tile([128, NPAD], BF, tag="t1")
                            nc.vector.tensor_scalar(
                                _rv(t1[:]), _rv(vb_sb[:]), fcol, None, OPS.mult,
                            )
                            t5 = epool.tile([128, NPAD], BF, tag="t5")
                            nc.vector.tensor_scalar(
                                _rv(t5[:]), _rv(v5b_sb[:]), f5col, None,
                                OPS.mult,
                            )
                            tM = epool.tile([128, NPAD], BF, tag="tM")
                            nc.vector.tensor_tensor(
                                _rv(tM[:]), _rv(t5[:]), _rv(t1[:]), OPS.max
                            )
                            nc.vector.tensor_tensor(
                                _rv(tA[:]), _rv(tM[:]), _rv(cntJ), OPS.mult,
                            )

                        if g == 0 and h == 0 and J == 0:
                            nc.sync.dma_start(_rv(d_dbg4), _rv(tA[:]))
                            nc.sync.dma_start(_rv(d_dbg5), _rv(vb_sb[:]))
                            nc.sync.dma_start(d_dbg6, c_fac[:])
                        base = J * BLKC + h * 33
                        for half in range(2):
                            sl = slice(half * GAP, half * GAP + NH)
                            nc.tensor.matmul(
                                oph[:, sl], xp33[:, base:base + 33], tA[:, sl],
                                start=(J == 0), stop=(J == NBLK - 1),
                            )

                    # stage_drain(h): free the single oph PSUM buffer
                    o_sb = dpool.tile([33, NPAD], FP, tag="o_sb")
                    nc.vector.tensor_copy(_rv(o_sb[:]), _rv(oph[:]))
                    if g == 0 and h == 0:
                        nc.sync.dma_start(_rv(d_dbg2), _rv(o_sb[:]))
                    den40 = dpool.tile([40, 25], FP, tag="den40")
                    for half in range(2):
                        nc.sync.dma_start(
                            den40[half * 20:(half + 1) * 20, :],
                            o_sb[32:33, half * GAP:half * GAP + NH],
                        )
                    pending[h] = {"o_sb": o_sb, "den40": den40}

                    if g == 0 and h == 1 and "recrow" in pending.get(0, {}):
                        pass
                    # stage_recip(h-1)
                    if h - 1 in pending:
                        st = pending[h - 1]
                        rec40 = dpool.tile([40, 25], BF, tag="rec40")
                        with nc.allow_low_precision(reason="bf16 rec ok"):
                            nc.vector.reciprocal(rec40[:], st["den40"][:])
                        recrow = dpool.tile([1, NPAD], BF, tag="recrow")
                        for half in range(2):
                            nc.sync.dma_start(
                                recrow[:, half * GAP:half * GAP + NH],
                                rec40[half * 20:(half + 1) * 20, :],
                            )
                        st["recrow"] = recrow
                        if g == 0 and h - 1 == 0:
                            nc.sync.dma_start(_rv(d_dbg3), _rv(recrow[:]))

                    # stage_finish(h-2)
                    if h - 2 in pending:
                        _emit_finish(nc, ps_misc, dpool, pending.pop(h - 2),
                                     c_onesb, c_bgat, c_pool, h - 2, g)

                # epilogue: flush heads 2 and 3
                st = pending[H - 1]
                rec40 = dpool.tile([40, 25], BF, tag="rec40")
                with nc.allow_low_precision(reason="bf16 rec ok"):
                    nc.vector.reciprocal(rec40[:], st["den40"][:])
                recrow = dpool.tile([1, NPAD], BF, tag="recrow")
                for half in range(2):
                    nc.sync.dma_start(
                        recrow[:, half * GAP:half * GAP + NH],
                        rec40[half * 20:(half + 1) * 20, :],
                    )
                st["recrow"] = recrow
                _emit_finish(nc, ps_misc, dpool, pending.pop(H - 2),
                             c_onesb, c_bgat, c_pool, H - 2, g)
                _emit_finish(nc, ps_misc, dpool, pending.pop(H - 1),
                             c_onesb, c_bgat, c_pool, H - 1, g)

            emit_lstm(G - 1)
            hp = lstm_state[0]
            ps3 = ps_misc.tile([OUT, 1], FP, tag="pm")
            nc.tensor.matmul(
                ps3[:], c_wclf[:], hp[:],
                start=True, stop=True,
            )
            ysb = lpool.tile([OUT, 1], FP, tag="ysb")
            nc.vector.tensor_tensor(ysb[:], ps3[:], c_bclf[:], OPS.add)
            nc.sync.dma_start(d_y, ysb[:])
            nc.sync.dma_start(d_dbg, c_pool[:])

    nc.compile()
    return nc


def _emit_finish(nc, ps_misc, dpool, st, c_onesb, c_bgat, c_pool, h, g):
    """rec-row broadcast + fused divide/maxpool + pooled write for head h."""
    o_sb, recrow = st["o_sb"], st["recrow"]
    scr = dpool.tile([32, NPAD], BF, tag="scr")
    macc = dpool.tile([32, 1], FP, tag="macc")
    for half in range(2):
        sl = slice(half * GAP, half * GAP + NH)
        rb = ps_misc.tile([32, GAP], FP, tag="pm")
        nc.tensor.matmul(
            rb[:, 0:NH], c_onesb[:, 0:32], recrow[:, sl],
            start=True, stop=True,
        )
        nc.vector.tensor_tensor(
            scr[:, sl], o_sb[0:32, sl], rb[:, 0:NH], OPS.mult
        )
    nc.vector.tensor_reduce(macc[:], _rv(scr[:]), AX.XY, OPS.max)
    pooled_h = dpool.tile([32, 1], FP, tag="pooled_h")
    nc.vector.tensor_scalar(
        pooled_h[:], macc[:], c_bgat[:, h:h + 1], 0.0,
        OPS.add, OPS.max,
    )
    nc.sync.dma_start(c_pool[h * 32:(h + 1) * 32, g:g + 1], pooled_h[:])


def _host_prep(inputs):
    x = np.asarray(inputs["x"], dtype=np.float32)          # [B, T, N, F]
    ei = np.asarray(inputs["edge_index"])
    W_gat = np.asarray(inputs["W_gat"], dtype=np.float32)  # [16, 128]
    att_src = np.asarray(inputs["att_src"], dtype=np.float32)  # [H, D]
    att_dst = np.asarray(inputs["att_dst"], dtype=np.float32)
    b_gat = np.asarray(inputs["b_gat"], dtype=np.float32)
    W_ih = np.asarray(inputs["W_ih"], dtype=np.float32)    # [256, 128]
    W_hh = np.asarray(inputs["W_hh"], dtype=np.float32)    # [256, 64]
    b_ih = np.asarray(inputs["b_ih"], dtype=np.float32)
    b_hh = np.asarray(inputs["b_hh"], dtype=np.float32)
    W_clf = np.asarray(inputs["W_clf"], dtype=np.float32)  # [8, 64]
    b_clf = np.asarray(inputs["b_clf"], dtype=np.float32)

    bf16 = mybir.dt.np(BF)

    def round_fr(x):
        u = np.ascontiguousarray(x, dtype=np.float32).view(np.uint32)
        r = ((u.astype(np.uint64) + 0x800) & 0xFFFFF000).astype(np.uint32)
        return r.view(np.float32)

    # fold attention vectors: a_s = x @ (W_gat-reshaped @ att_src)
    Wr = W_gat.reshape(F_IN, H, D)
    W_as = np.einsum("fhd,hd->fh", Wr, att_src)            # [16, 4]
    W_ad = np.einsum("fhd,hd->fh", Wr, att_dst)
    w_all = np.concatenate([W_gat, W_as], axis=1)          # [16, 132]

    # edge counts with self loops; gap layout [128, 8*(512+512)],
    # halves hold real cols 0:500 / 500:1000 at offsets 0 / 512
    src = ei[0].astype(np.int64)
    dst = ei[1].astype(np.int64)
    Cm = np.zeros((NPAD, N), dtype=np.float32)
    np.add.at(Cm, (src, dst), 1.0)
    Cm[np.arange(N), np.arange(N)] += 1.0                  # self loops
    cnt4 = np.zeros((NBLK, 128, 2, GAP), dtype=np.float32)
    CmJ = Cm.reshape(NBLK, 128, N)
    cnt4[:, :, 0, 0:NH] = CmJ[:, :, 0:NH]
    cnt4[:, :, 1, 0:NH] = CmJ[:, :, NH:N]
    cntmask = (
        cnt4.reshape(NBLK, 128, NPAD).transpose(1, 0, 2).reshape(128, NBLK * NPAD)
    ).astype(bf16)

    # x pre-transposed per core: [F, G*NPAD]
    xpad = np.zeros((B, T, NPAD, F_IN), dtype=np.float32)
    xpad[:, :, :N, :] = x
    xts = [
        round_fr(xpad[b].transpose(2, 0, 1).reshape(F_IN, T * NPAD))
        for b in range(B)
    ]

    b_gates = (b_ih + b_hh).astype(np.float32)             # [256]
    bls = np.zeros((2 * HL, 2), dtype=np.float32)
    bls[:, 0] = 0.5 * b_gates[0:128]                       # i, f (tanh trick)
    bls[0:HL, 1] = b_gates[128:192]                        # g
    bls[HL:2 * HL, 1] = 0.5 * b_gates[192:256]             # o
    scl23 = np.zeros((2 * HL, 1), dtype=np.float32)
    scl23[0:HL, 0] = 1.0
    scl23[HL:2 * HL, 0] = 0.5

    common = {
        "w_all": round_fr(w_all),
        "w_ad": round_fr(W_ad),
        "cntmask": cntmask,
        "ones_bf": np.ones((1, 128), dtype=bf16),
        "ones_fp": np.ones((1, 128), dtype=np.float32),
        "b_gat": np.ascontiguousarray(b_gat.reshape(H, 32).T),
        "wih01": np.ascontiguousarray(W_ih[0:128, :].T),       # [128, 128]
        "wih23": np.ascontiguousarray(W_ih[128:256, :].T),
        "whh01": np.ascontiguousarray(0.5 * W_hh[0:128, :].T),  # [64, 128]
        "whh23": np.ascontiguousarray(0.5 * W_hh[128:256, :].T),
        "b_lstm": bls,
        "scl23": scl23,
        "wclf_t": np.ascontiguousarray(0.5 * W_clf.T),     # [64, 8] (h2 comp)
        "b_clf": b_clf.reshape(OUT, 1),
    }
    in_maps = []
    for b in range(B):
        m = dict(common)
        m["x_t"] = xts[b]
        in_maps.append(m)
    return in_maps


def kernel(**inputs):
    if "nc" not in _CACHE:
        _CACHE["nc"] = _build_nc()
    nc = _CACHE["nc"]
    in_maps = _host_prep(inputs)
    res = run_bass_kernel_spmd(nc, in_maps, core_ids=list(range(B)))
    y = np.stack([r["y"][:, 0] for r in res.results], axis=0)
    return y.astype(np.float32)


if __name__ == "__main__":
    import reference as R

    inp = R.setup_inputs()
    inp = {k: np.asarray(v) for k, v in inp.items()}
    out = kernel(**inp)
    print(out)


# revision 39
# speedup vs baseline: 1.0534x; 1.0534x over previous
"""GAT + global-max-pool + LSTM + Linear kernel for Trainium2 (8 NeuronCores).

Sharding: data-parallel over the batch axis B=8 -> one sequence b per core.
Each core computes the GAT over its 16 graphs (t=0..15), global-max-pools,
runs the LSTM over its sequence locally, and emits one [8] output row.

v2 redesign (engine-balanced):
  - host pre-transposes x -> xT [16, G*1024]; per-graph DMA load (no PE
    transposes on device).
  - one merged fp32r matmul per 128-src-node block J computes xp|a_src
    directly in [m, hd] layout (no xpT + transpose round trip).
  - per head: raw a_dst row broadcast once to 128 partitions (2 fp32r
    matmuls -> adB PSUM); vb_sb = Exp(adB), v5b_sb = Exp(0.2 adB) SBUF
    mirrors feed the DVE fast paths.
  - dense attention tile per (head h, src block J), engine-split:
      A-path (small J): t1 = ACT Exp(adB + a_s), t5 = ACT Exp(.2 adB + .2 a_s),
                        tM = DVE tt-max
      B-path: tE = DVE ts(vb_sb * u) in 4x mode; tM via Pool fused stt or
              DVE ts+tt
      tA = tM * cnt on DVE (2x bf16) or Pool
      agg: oph[33, n] += xp33^T @ tA   (bf16 matmuls, 500-col halves)
  - divide+relu+maxpool fused and software-pipelined two heads deep:
    Pool copies oph -> SBUF (frees the single PSUM buffer), DMA folds the
    denominator row to [40,25], DVE reciprocal, DMA unfold, bf16 PE
    broadcast of rec row, tensor_tensor_reduce chains the max over both
    halves; b_gat applied after the reduce (exact: b is constant over
    nodes).
  - LSTM step g emitted inside graph g+1 (hidden behind its factor
    stage). Gates packed 2-per-matmul, tanh-sigmoid trick with
    per-partition scale columns.
  - real 1000 cols only, gap layout (halves at col offsets 0 and 512).

softmax max-subtraction is dropped: alpha = ex/sum(ex) is invariant to the
per-dst shift and fp32/bf16 exp() of |z| <~ 10 cannot overflow.
"""

import numpy as np

import concourse.bacc as bacc
import concourse.bass as bass
import concourse.mybir as mybir
import concourse.tile as tile
from concourse.bass_utils import run_bass_kernel_spmd

B, T, N, F_IN = 8, 16, 1000, 16
H, D = 4, 32
HD = H * D          # 128
HL = 64
OUT = 8
NEG = 0.2
NPAD = 1024         # padded node count
NBLK = 8            # src blocks of 128
G = T               # graphs per core
NH = 500            # real cols per half
GAP = 512           # col offset stride of halves
BLKC = H * 33 + 4   # xp33 cols per src block (4 head groups + pad)

FP = mybir.dt.float32
FR = mybir.dt.float32r
BF = mybir.dt.bfloat16
AX = mybir.AxisListType
AF = mybir.ActivationFunctionType
OPS = mybir.AluOpType

_CACHE = {}

# engine assignment knobs, per src block J (same for all heads)
A_PATH_J = 0          # J < this: t1/t5 via ACT Exp-with-bias
P2_POOL_J = ()  # fused stt on Pool
P3_POOL_J = ()     # cnt-mult on Pool


def _rv(ap):
    """Real-column view [p, 2, 500] of a gap-layout [p, 1024] AP."""
    return ap.rearrange("p (a b) -> p a b", b=GAP)[:, :, 0:NH]


def _build_nc():
    nc = bacc.Bacc("TRN2", target_bir_lowering=False, debug=False)

    # ---- DRAM I/O ----
    d_xT = nc.dram_tensor("x_t", [F_IN, G * NPAD], FR, kind="ExternalInput").ap()
    d_wall = nc.dram_tensor("w_all", [F_IN, HD + H], FR, kind="ExternalInput").ap()
    d_wad = nc.dram_tensor("w_ad", [F_IN, H], FR, kind="ExternalInput").ap()
    d_cnt = nc.dram_tensor("cntmask", [128, NBLK * NPAD], BF, kind="ExternalInput").ap()
    d_onesb = nc.dram_tensor("ones_bf", [1, 128], BF, kind="ExternalInput").ap()
    d_onesf = nc.dram_tensor("ones_fp", [1, 128], FR, kind="ExternalInput").ap()
    d_bgat = nc.dram_tensor("b_gat", [32, H], FP, kind="ExternalInput").ap()
    d_wih01 = nc.dram_tensor("wih01", [HD, 2 * HL], FP, kind="ExternalInput").ap()
    d_wih23 = nc.dram_tensor("wih23", [HD, 2 * HL], FP, kind="ExternalInput").ap()
    d_whh01 = nc.dram_tensor("whh01", [HL, 2 * HL], FP, kind="ExternalInput").ap()
    d_whh23 = nc.dram_tensor("whh23", [HL, 2 * HL], FP, kind="ExternalInput").ap()
    d_bls = nc.dram_tensor("b_lstm", [2 * HL, 2], FP, kind="ExternalInput").ap()
    d_scl23 = nc.dram_tensor("scl23", [2 * HL, 1], FP, kind="ExternalInput").ap()
    d_wclf = nc.dram_tensor("wclf_t", [HL, OUT], FP, kind="ExternalInput").ap()
    d_bclf = nc.dram_tensor("b_clf", [OUT, 1], FP, kind="ExternalInput").ap()
    d_y = nc.dram_tensor("y", [OUT, 1], FP, kind="ExternalOutput").ap()
    d_dbg = nc.dram_tensor("dbg_pool", [HD, G], FP, kind="ExternalOutput").ap()
    d_dbg2 = nc.dram_tensor("dbg_osb", [33, NPAD], FP, kind="ExternalOutput").ap()
    d_dbg3 = nc.dram_tensor("dbg_rec", [1, NPAD], BF, kind="ExternalOutput").ap()
    d_dbg4 = nc.dram_tensor("dbg_tA", [128, NPAD], BF, kind="ExternalOutput").ap()
    d_dbg5 = nc.dram_tensor("dbg_vb", [128, NPAD], BF, kind="ExternalOutput").ap()
    d_dbg6 = nc.dram_tensor("dbg_fac", [128, NBLK * 8], FP, kind="ExternalOutput").ap()

    with tile.TileContext(nc) as tc:
        with (
            tc.tile_pool(name="const", bufs=1) as cpool,
            tc.tile_pool(name="xtp", bufs=2) as xtp,
            tc.tile_pool(name="fact", bufs=2) as fpool,
            tc.tile_pool(name="bcs", bufs=2) as bpool,
            tc.tile_pool(name="edense", bufs=3) as epool,
            tc.tile_pool(name="divp", bufs=3) as dpool,
            tc.tile_pool(name="lstm", bufs=2) as lpool,
            tc.tile_pool(name="ps_bc", bufs=1, space="PSUM") as ps_bc,
            tc.tile_pool(name="ps_out", bufs=2, space="PSUM") as ps_out,
            tc.tile_pool(name="ps_misc", bufs=2, space="PSUM") as ps_misc,
        ):
            # ---- load constants ----
            c_wall = cpool.tile([F_IN, HD + H], FR, tag="wall")
            nc.sync.dma_start(c_wall[:], d_wall)
            c_wad = cpool.tile([F_IN, H], FR, tag="wad")
            nc.sync.dma_start(c_wad[:], d_wad)
            c_cnt = cpool.tile([128, NBLK * NPAD], BF, tag="cnt")
            nc.sync.dma_start(c_cnt[:], d_cnt)
            c_onesb = cpool.tile([1, 128], BF, tag="onesb")
            nc.sync.dma_start(c_onesb[:], d_onesb)
            c_onesf = cpool.tile([1, 128], FR, tag="onesf")
            nc.sync.dma_start(c_onesf[:], d_onesf)
            c_bgat = cpool.tile([32, H], FP, tag="bgat")
            nc.sync.dma_start(c_bgat[:], d_bgat)
            c_wih01 = cpool.tile([HD, 2 * HL], FP, tag="wih01")
            nc.sync.dma_start(c_wih01[:], d_wih01)
            c_wih23 = cpool.tile([HD, 2 * HL], FP, tag="wih23")
            nc.sync.dma_start(c_wih23[:], d_wih23)
            c_whh01 = cpool.tile([HL, 2 * HL], FP, tag="whh01")
            nc.sync.dma_start(c_whh01[:], d_whh01)
            c_whh23 = cpool.tile([HL, 2 * HL], FP, tag="whh23")
            nc.sync.dma_start(c_whh23[:], d_whh23)
            c_bls = cpool.tile([2 * HL, 2], FP, tag="bls")
            nc.sync.dma_start(c_bls[:], d_bls)
            c_scl23 = cpool.tile([2 * HL, 1], FP, tag="scl23")
            nc.sync.dma_start(c_scl23[:], d_scl23)
            c_wclf = cpool.tile([HL, OUT], FP, tag="wclf")
            nc.sync.dma_start(c_wclf[:], d_wclf)
            c_bclf = cpool.tile([OUT, 1], FP, tag="bclf")
            nc.sync.dma_start(c_bclf[:], d_bclf)

            # persistent: pooled sequence + manually double-buffered xp33
            c_pool = cpool.tile([HD, G], FP, tag="pooled")
            xp33s = []
            for i in range(2):
                xp_t = cpool.tile([128, NBLK * BLKC], BF, tag=f"xp33_{i}")
                for J in range(NBLK):
                    nc.vector.memset(
                        xp_t[:, J * BLKC:J * BLKC + H * 33].rearrange(
                            "p (h q) -> p h q", q=33
                        )[:, :, 32:33],
                        1.0,
                    )
                xp33s.append(xp_t)

            hprev = lpool.tile([HL, 1], FP, tag="h0")
            cprev = lpool.tile([HL, 1], FP, tag="c0")
            nc.vector.memset(hprev[:], 0.0)
            nc.vector.memset(cprev[:], 0.0)
            lstm_state = [hprev, cprev]

            def emit_lstm(g):
                hp, cp = lstm_state
                psg01 = ps_misc.tile([2 * HL, 1], FP, tag="pm")
                nc.tensor.matmul(
                    psg01[:], c_wih01[:],
                    c_pool[:, g:g + 1], start=True, stop=False,
                )
                nc.tensor.matmul(
                    psg01[:], c_whh01[:], hp[:],
                    start=False, stop=True,
                )
                psg23 = ps_misc.tile([2 * HL, 1], FP, tag="pm")
                nc.tensor.matmul(
                    psg23[:], c_wih23[:],
                    c_pool[:, g:g + 1], start=True, stop=False,
                )
                nc.tensor.matmul(
                    psg23[:], c_whh23[:], hp[:],
                    start=False, stop=True,
                )
                tg01 = lpool.tile([2 * HL, 1], FP, tag="tg01")
                nc.scalar.activation(
                    tg01[:], psg01[:], AF.Tanh, bias=c_bls[:, 0:1], scale=0.5,
                )
                tg23 = lpool.tile([2 * HL, 1], FP, tag="tg23")
                nc.scalar.activation(
                    tg23[:], psg23[:], AF.Tanh, bias=c_bls[:, 1:2],
                    scale=c_scl23[:, 0:1],
                )
                tf0 = lpool.tile([HL, 1], FP, tag="tf0")
                nc.sync.dma_start(tf0[:], tg01[HL:2 * HL, :])
                to0 = lpool.tile([HL, 1], FP, tag="to0")
                nc.sync.dma_start(to0[:], tg23[HL:2 * HL, :])
                # v1 = (tf+1)*c2 ; v2 = (ti+1)*tg ; c2' = v1/2 + v2
                v1 = lpool.tile([HL, 1], FP, tag="v1")
                nc.vector.scalar_tensor_tensor(
                    v1[:], tf0[:], 1.0, cp[:], OPS.add, OPS.mult
                )
                v2 = lpool.tile([HL, 1], FP, tag="v2")
                nc.vector.scalar_tensor_tensor(
                    v2[:], tg01[0:HL, :], 1.0, tg23[0:HL, :], OPS.add, OPS.mult
                )
                cnew = lpool.tile([HL, 1], FP, tag="c0")
                nc.vector.scalar_tensor_tensor(
                    cnew[:], v1[:], 0.5, v2[:], OPS.mult, OPS.add
                )
                tcn = lpool.tile([HL, 1], FP, tag="tcn")
                nc.scalar.activation(tcn[:], cnew[:], AF.Tanh, scale=0.5)
                hnew = lpool.tile([HL, 1], FP, tag="h0")
                nc.vector.scalar_tensor_tensor(
                    hnew[:], to0[:], 1.0, tcn[:], OPS.add, OPS.mult
                )
                lstm_state[0], lstm_state[1] = hnew, cnew

            def emit_bcast(ad_row, h):
                """Broadcast a_d row h to 128 partitions; SBUF mirrors."""
                adB = ps_bc.tile([128, NPAD], FP, tag="adB")
                for half in range(2):
                    sl = slice(half * GAP, half * GAP + NH)
                    nc.tensor.matmul(
                        adB[:, sl], c_onesf[:],
                        ad_row[:, sl],
                        start=True, stop=True,
                    )
                vb_sb = bpool.tile([128, NPAD], BF, tag=f"vb_sb{h}")
                nc.scalar.activation(_rv(vb_sb[:]), _rv(adB[:]), AF.Exp)
                v5b_sb = bpool.tile([128, NPAD], BF, tag=f"v5b_sb{h}")
                nc.scalar.activation(_rv(v5b_sb[:]), _rv(adB[:]), AF.Exp, scale=NEG)
                return adB, vb_sb, v5b_sb

            for g in range(G):
                xp33 = xp33s[g % 2]
                # ---- load xT for this graph ----
                xT = xtp.tile([F_IN, NPAD], FR, tag="xT")
                nc.sync.dma_start(xT[:], d_xT[:, g * NPAD:(g + 1) * NPAD])
                xTr = xT[:]

                # ---- a_dst rows (raw, fp32); one [1, NPAD] tile per head
                # (matmul rhs requires base partition 0) ----
                ad_sb = fpool.tile([H, NPAD], FR, tag="ad_sb")
                for half in range(2):
                    pad_ = ps_misc.tile([H, GAP], FP, tag="pm")
                    nc.tensor.matmul(
                        pad_[:, 0:NH], c_wad[:],
                        xTr[:, half * NH:half * NH + NH],
                        start=True, stop=True,
                    )
                    nc.scalar.activation(
                        ad_sb[:, half * GAP:half * GAP + NH],
                        pad_[:, 0:NH], AF.Copy,
                    )
                nc.vector.memset(ad_sb[:, NH:GAP].bitcast(mybir.dt.uint32), 0)
                nc.vector.memset(ad_sb[:, GAP + NH:NPAD].bitcast(mybir.dt.uint32), 0)
                ad_rows = []
                for h in range(H):
                    ad_h = fpool.tile([1, NPAD], FR, tag=f"ad_h{h}")
                    nc.sync.dma_start(ad_h[:], ad_sb[h:h + 1, :])
                    ad_rows.append(ad_h)

                # ---- per src block: xp | a_src factors, plus all four
                # head broadcasts interleaved to keep the PE streaming ----
                # c_fac cols per J: [0:4] = a_s (A-path) or exp(a_s) (B-path)
                #                   [4:8] = 0.2*a_s or exp(0.2*a_s)
                c_fac = fpool.tile([128, NBLK * 8], FP, tag="c_fac")
                bcast = [None] * H

                def emit_pxa(J):
                    pxa = ps_misc.tile([128, HD + H], FP, tag="pm")
                    nc.tensor.matmul(
                        pxa[:], xTr[:, J * 128:(J + 1) * 128],
                        c_wall[:],
                        start=True, stop=True,
                    )
                    base = J * BLKC
                    nc.vector.tensor_copy(
                        xp33[:, base:base + H * 33].rearrange(
                            "p (h q) -> p h q", q=33
                        )[:, :, 0:32],
                        pxa[:, 0:HD].rearrange("p (h q) -> p h q", q=32),
                    )
                    if J < PRELU_J:
                        nc.vector.tensor_copy(
                            c_fac[:, J * 8:J * 8 + 4], pxa[:, HD:HD + H],
                        )
                    else:
                        nc.scalar.activation(
                            c_fac[:, J * 8:J * 8 + 4], pxa[:, HD:HD + H],
                            AF.Exp, scale=1.0,
                        )
                        nc.scalar.activation(
                            c_fac[:, J * 8 + 4:J * 8 + 8], pxa[:, HD:HD + H],
                            AF.Exp, scale=NEG,
                        )

                emit_pxa(0)
                emit_pxa(1)
                for h in range(H):
                    bcast[h] = emit_bcast(ad_rows[h], h)
                    if 2 + h < NBLK:
                        emit_pxa(2 + h)
                emit_pxa(6)
                emit_pxa(7)
                if g > 0:
                    emit_lstm(g - 1)

                # ---- heads: software-pipelined divide path ----
                # stage state carried across head iterations
                pending = {}   # h -> dict of tiles for deferred stages
                for h in range(H):
                    adB_ps, vb_sb, v5b_sb = bcast[h]

                    oph = ps_out.tile([33, NPAD], FP, tag="oph")
                    for J in range(NBLK):
                        fcol = c_fac[:, J * 8 + h:J * 8 + h + 1]
                        f5col = c_fac[:, J * 8 + 4 + h:J * 8 + 5 + h]
                        cntJ = c_cnt[:, J * NPAD:(J + 1) * NPAD]
                        tA = epool.tile([128, NPAD], BF, tag="tA")
                        if J < PRELU_J:
                            # w = lrelu(a_d[n] + a_s[m]); tM = exp(w)
                            tw = epool.tile([128, NPAD], mybir.dt.float16,
                                            tag="tw")
                            nc.scalar.activation(
                                _rv(tw[:]), _rv(adB_ps[:]), AF.Prelu,
                                bias=fcol, scale=1.0, alpha=NEG,
                            )
                            tM = epool.tile([128, NPAD], BF, tag="tM")
                            nc.scalar.activation(
                                _rv(tM[:]), _rv(tw[:]), AF.Exp,
                            )
                            eng = nc.gpsimd if J in P3_POOL_J else nc.vector
                            eng.tensor_tensor(
                                _rv(tA[:]), _rv(tM[:]), _rv(cntJ), OPS.mult,
                            )
                        else:
                            t1 = epool.tile([128, NPAD], BF, tag="t1")
                            nc.vector.tensor_scalar(
                                _rv(t1[:]), _rv(vb_sb[:]), fcol, None, OPS.mult,
                            )
                            t5 = epool.tile([128, NPAD], BF, tag="t5")
                            nc.vector.tensor_scalar(
                                _rv(t5[:]), _rv(v5b_sb[:]), f5col, None,
                                OPS.mult,
                            )
                            tM = epool.tile([128, NPAD], BF, tag="tM")
                            nc.vector.tensor_tensor(
                                _rv(tM[:]), _rv(t5[:]), _rv(t1[:]), OPS.max
                            )
                            nc.vector.tensor_tensor(
                                _rv(tA[:]), _rv(tM[:]), _rv(cntJ), OPS.mult,
                            )

                        if g == 0 and h == 0 and J == 0:
                            nc.sync.dma_start(_rv(d_dbg4), _rv(tA[:]))
                            nc.sync.dma_start(_rv(d_dbg5), _rv(vb_sb[:]))
                            nc.sync.dma_start(d_dbg6, c_fac[:])
                        base = J * BLKC + h * 33
                        for half in range(2):
                            sl = slice(half * GAP, half * GAP + NH)
                            nc.tensor.matmul(
                                oph[:, sl], xp33[:, base:base + 33], tA[:, sl],
                                start=(J == 0), stop=(J == NBLK - 1),
                            )

                    # stage_drain(h): free the single oph PSUM buffer
                    o_sb = dpool.tile([33, NPAD], FP, tag="o_sb")
                    nc.vector.tensor_copy(_rv(o_sb[:]), _rv(oph[:]))
                    if g == 0 and h == 0:
                        nc.sync.dma_start(_rv(d_dbg2), _rv(o_sb[:]))
                    den40 = dpool.tile([40, 25], FP, tag="den40")
                    for half in range(2):
                        nc.sync.dma_start(
                            den40[half * 20:(half + 1) * 20, :],
                            o_sb[32:33, half * GAP:half * GAP + NH],
                        )
                    pending[h] = {"o_sb": o_sb, "den40": den40}

                    if g == 0 and h == 1 and "recrow" in pending.get(0, {}):
                        pass
                    # stage_recip(h-1)
                    if h - 1 in pending:
                        st = pending[h - 1]
                        rec40 = dpool.tile([40, 25], BF, tag="rec40")
                        with nc.allow_low_precision(reason="bf16 rec ok"):
                            nc.vector.reciprocal(rec40[:], st["den40"][:])
                        recrow = dpool.tile([1, NPAD], BF, tag="recrow")
                        for half in range(2):
                            nc.sync.dma_start(
                                recrow[:, half * GAP:half * GAP + NH],
                                rec40[half * 20:(half + 1) * 20, :],
                            )
                        st["recrow"] = recrow
                        if g == 0 and h - 1 == 0:
                            nc.sync.dma_start(_rv(d_dbg3), _rv(recrow[:]))

                    # stage_finish(h-2)
                    if h - 2 in pending:
                        _emit_finish(nc, ps_misc, dpool, pending.pop(h - 2),
                                     c_onesb, c_bgat, c_pool, h - 2, g)

                # epilogue: flush heads 2 and 3
                st = pending[H - 1]
                rec40 = dpool.tile([40, 25], BF, tag="rec40")
                with nc.allow_low_precision(reason="bf16 rec ok"):
                    nc.vector.reciprocal(rec40[:], st["den40"][:])
                recrow = dpool.tile([1, NPAD], BF, tag="recrow")
                for half in range(2):
                    nc.sync.dma_start(
                        recrow[:, half * GAP:half * GAP + NH],
                        rec40[half * 20:(half + 1) * 20, :],
                    )
                st["recrow"] = recrow
                _emit_finish(nc, ps_misc, dpool, pending.pop(H - 2),
                             c_onesb, c_bgat, c_pool, H - 2, g)
                _emit_finish(nc, ps_misc, dpool, pending.pop(H - 1),
                             c_onesb, c_bgat, c_pool, H - 1, g)

            emit_lstm(G - 1)
            hp = lstm_state[0]
            ps3 = ps_misc.tile([OUT, 1], FP, tag="pm")
            nc.tensor.matmul(
                ps3[:], c_wclf[:], hp[:],
                start=True, stop=True,
            )
            ysb = lpool.tile([OUT, 1], FP, tag="ysb")
            nc.vector.tensor_tensor(ysb[:], ps3[:], c_bclf[:], OPS.add)
            nc.sync.dma_start(d_y, ysb[:])
            nc.sync.dma_start(d_dbg, c_pool[:])

    nc.compile()
    return nc


def _emit_finish(nc, ps_misc, dpool, st, c_onesb, c_bgat, c_pool, h, g):
    """rec-row broadcast + fused divide/maxpool + pooled write for head h."""
    o_sb, recrow = st["o_sb"], st["recrow"]
    scr = dpool.tile([32, NPAD], BF, tag="scr")
    macc = dpool.tile([32, 1], FP, tag="macc")
    for half in range(2):
        sl = slice(half * GAP, half * GAP + NH)
        rb = ps_misc.tile([32, GAP], FP, tag="pm")
        nc.tensor.matmul(
            rb[:, 0:NH], c_onesb[:, 0:32], recrow[:, sl],
            start=True, stop=True,
        )
        nc.vector.tensor_tensor(
            scr[:, sl], o_sb[0:32, sl], rb[:, 0:NH], OPS.mult
        )
    nc.vector.tensor_reduce(macc[:], _rv(scr[:]), AX.XY, OPS.max)
    pooled_h = dpool.tile([32, 1], FP, tag="pooled_h")
    nc.vector.tensor_scalar(
        pooled_h[:], macc[:], c_bgat[:, h:h + 1], 0.0,
        OPS.add, OPS.max,
    )
    nc.sync.dma_start(c_pool[h * 32:(h + 1) * 32, g:g + 1], pooled_h[:])


def _host_prep(inputs):
    x = np.asarray(inputs["x"], dtype=np.float32)          # [B, T, N, F]
    ei = np.asarray(inputs["edge_index"])
    W_gat = np.asarray(inputs["W_gat"], dtype=np.float32)  # [16, 128]
    att_src = np.asarray(inputs["att_src"], dtype=np.float32)  # [H, D]
    att_dst = np.asarray(inputs["att_dst"], dtype=np.float32)
    b_gat = np.asarray(inputs["b_gat"], dtype=np.float32)
    W_ih = np.asarray(inputs["W_ih"], dtype=np.float32)    # [256, 128]
    W_hh = np.asarray(inputs["W_hh"], dtype=np.float32)    # [256, 64]
    b_ih = np.asarray(inputs["b_ih"], dtype=np.float32)
    b_hh = np.asarray(inputs["b_hh"], dtype=np.float32)
    W_clf = np.asarray(inputs["W_clf"], dtype=np.float32)  # [8, 64]
    b_clf = np.asarray(inputs["b_clf"], dtype=np.float32)

    bf16 = mybir.dt.np(BF)

    def round_fr(x):
        u = np.ascontiguousarray(x, dtype=np.float32).view(np.uint32)
        r = ((u.astype(np.uint64) + 0x800) & 0xFFFFF000).astype(np.uint32)
        return r.view(np.float32)

    # fold attention vectors: a_s = x @ (W_gat-reshaped @ att_src)
    Wr = W_gat.reshape(F_IN, H, D)
    W_as = np.einsum("fhd,hd->fh", Wr, att_src)            # [16, 4]
    W_ad = np.einsum("fhd,hd->fh", Wr, att_dst)
    w_all = np.concatenate([W_gat, W_as], axis=1)          # [16, 132]

    # edge counts with self loops; gap layout [128, 8*(512+512)],
    # halves hold real cols 0:500 / 500:1000 at offsets 0 / 512
    src = ei[0].astype(np.int64)
    dst = ei[1].astype(np.int64)
    Cm = np.zeros((NPAD, N), dtype=np.float32)
    np.add.at(Cm, (src, dst), 1.0)
    Cm[np.arange(N), np.arange(N)] += 1.0                  # self loops
    cnt4 = np.zeros((NBLK, 128, 2, GAP), dtype=np.float32)
    CmJ = Cm.reshape(NBLK, 128, N)
    cnt4[:, :, 0, 0:NH] = CmJ[:, :, 0:NH]
    cnt4[:, :, 1, 0:NH] = CmJ[:, :, NH:N]
    cntmask = (
        cnt4.reshape(NBLK, 128, NPAD).transpose(1, 0, 2).reshape(128, NBLK * NPAD)
    ).astype(bf16)

    # x pre-transposed per core: [F, G*NPAD]
    xpad = np.zeros((B, T, NPAD, F_IN), dtype=np.float32)
    xpad[:, :, :N, :] = x
    xts = [
        round_fr(xpad[b].transpose(2, 0, 1).reshape(F_IN, T * NPAD))
        for b in range(B)
    ]

    b_gates = (b_ih + b_hh).astype(np.float32)             # [256]
    bls = np.zeros((2 * HL, 2), dtype=np.float32)
    bls[:, 0] = 0.5 * b_gates[0:128]                       # i, f (tanh trick)
    bls[0:HL, 1] = b_gates[128:192]                        # g
    bls[HL:2 * HL, 1] = 0.5 * b_gates[192:256]             # o
    scl23 = np.zeros((2 * HL, 1), dtype=np.float32)
    scl23[0:HL, 0] = 1.0
    scl23[HL:2 * HL, 0] = 0.5

    common = {
        "w_all": round_fr(w_all),
        "w_ad": round_fr(W_ad),
        "cntmask": cntmask,
        "ones_bf": np.ones((1, 128), dtype=bf16),
        "ones_fp": np.ones((1, 128), dtype=np.float32),
        "b_gat": np.ascontiguousarray(b_gat.reshape(H, 32).T),
        "wih01": np.ascontiguousarray(W_ih[0:128, :].T),       # [128, 128]
        "wih23": np.ascontiguousarray(W_ih[128:256, :].T),
        "whh01": np.ascontiguousarray(0.5 * W_hh[0:128, :].T),  # [64, 128]
        "whh23": np.ascontiguousarray(0.5 * W_hh[128:256, :].T),
        "b_lstm": bls,
        "scl23": scl23,
        "wclf_t": np.ascontiguousarray(0.5 * W_clf.T),     # [64, 8] (h2 comp)
        "b_clf": b_clf.reshape(OUT, 1),
    }
    in_maps = []
    for b in range(B):
        m = dict(common)
        m["x_t"] = xts[b]
        in_maps.append(m)
    return in_maps


def kernel(**inputs):
    if "nc" not in _CACHE:
        _CACHE["nc"] = _build_nc()
    nc = _CACHE["nc"]
    in_maps = _host_prep(inputs)
    res = run_bass_kernel_spmd(nc, in_maps, core_ids=list(range(B)))
    y = np.stack([r["y"][:, 0] for r in res.results], axis=0)
    return y.astype(np.float32)


if __name__ == "__main__":
    import reference as R

    inp = R.setup_inputs()
    inp = {k: np.asarray(v) for k, v in inp.items()}
    out = kernel(**inp)
    print(out)
